# revision 29
# baseline (speedup 1.0000x reference)
"""Trainium2 Bass kernel for nn_DenoisingSharpening (v3, wrap-flat layout).

Contract: kernel(**inputs) takes the FULL unsharded inputs
(images [8,64,64,64,3] f32, params [8,64,7] f32, k [] f32) and returns
the FULL output [8,64,64,64,3] f32.

Strategy
--------
Data-parallel over N = B*P = 512 images; 64 images per NeuronCore, one
half-image (32 rows) per SBUF partition -> 128 partitions x 8 cores.

Real TRN2 engines pay a large per-segment tax on multi-row strided
access patterns, so every op here works on FLAT per-channel-plane spans
("wrap" layout): a +-1 row/col stencil shift is just a +-1/+-66 flat
offset.  Flat-offset neighbor arithmetic is exact for interior columns;
the two pad columns per row carry wrap garbage which never feeds an
interior lane (checked per tap) and is stripped on the host.  The two
pad columns of `inner` are zeroed so the skip-mean accumulators stay
exact.

Per chunk of CR rows: 4 bilateral tap fields (W, NW, N, NE) as flat
diffs of the planar f16 input; ck = exp(-(s*d)^2 sum + log w) with
squares on ACT and the exp batched per weight class; numerator via the
symmetric-pair trick dif = prod_I - prod_M; denominator from flat ck
adds; bf - x = nacc/sacc via reciprocal_approx_fast; separable gaussian
detail rebuilt from the same dW/dN fields; noise chain on ACT (single
table set: Square/Exp/Abs/Tanh/Identity/Copy); skip decision on host
from shipped per-chunk sums.
"""

import numpy as np

N_CORES = 8
B, PP, H, W, C = 8, 64, 64, 64, 3
NIMG = B * PP          # 512
HALVES = 2 * NIMG      # 1024 half-images, 128 per core
PR, PC = 34, 66        # padded half-image rows/cols
ROWS_PER_HALF = 32
PLANE = PR * PC        # 2244 flat elems per channel plane
FPAD = 68              # front pad per plane (worst back-read -67)
BPAD = 4               # back pad (H reads +2 past slab end)
PLANE_T = FPAD + PLANE + BPAD   # 2316 per-plane tile pitch

NOISE_THRESH = 0.002
SKIP_THRESH = 1e-4
MEAN_N = float(C * H * W)

# params columns
(P_S, P_LOGE, P_LOGC, P_WSC, P_BE, P_1M2BE, P_IGT, P_OFFGT, P_CLIP,
 P_KT, P_KTB, P_SQL2, P_NSQL2, P_NS2) = range(14)
NPARAM = 16

_CACHE = {}


# --------------------------------------------------------------------------
# host-side preprocessing
# --------------------------------------------------------------------------

def _host_prep(images, params, k):
    x = np.ascontiguousarray(images, dtype=np.float32).reshape(NIMG, H, W, C)
    xp = np.pad(x, ((0, 0), (1, 1), (1, 1), (0, 0)), mode="reflect")
    halves = np.stack([xp[:, 0:PR], xp[:, ROWS_PER_HALF:ROWS_PER_HALF + PR]],
                      axis=1).reshape(HALVES, PR, PC, C)
    planar = np.ascontiguousarray(
        halves.transpose(0, 3, 1, 2), dtype=np.float16).reshape(
            HALVES, C, PLANE)

    p = np.asarray(params, dtype=np.float32).reshape(NIMG, 7)
    sigma_r = np.clip(p[:, 1], 0.01, 1.0)
    sigma_s = np.clip(p[:, 0], 0.2, 5.0)
    sigma_f = np.clip(p[:, 2], 0.2, 3.0)
    lam = np.clip(p[:, 3], 0.1, 2.0)
    tau = np.clip(p[:, 4], 0.5, 5.0)
    gain = np.clip(p[:, 5], 0.2, 2.0)
    offset = np.clip(p[:, 6], 0.01, 1.0)

    def gauss1d(sig):
        g = np.exp(-0.5 * (np.array([-1.0, 0.0, 1.0], np.float32)[None, :]
                           / sig[:, None]) ** 2)
        return g / g.sum(axis=1, keepdims=True)

    gs = gauss1d(sigma_s)
    gf = gauss1d(sigma_f)
    aE, aC = gs[:, 0], gs[:, 1]
    bE = gf[:, 0]

    kpos = max(abs(float(np.asarray(k))), 1.0)
    gt = gain / tau
    sql2 = np.sqrt(lam * bE)

    pars = np.zeros((NIMG, NPARAM), np.float32)
    pars[:, P_S] = np.sqrt(0.5) / sigma_r
    pars[:, P_LOGE] = np.log(aE * aC)
    pars[:, P_LOGC] = np.log(aE * aE)
    pars[:, P_WSC] = aC * aC
    pars[:, P_BE] = bE
    pars[:, P_1M2BE] = 1.0 - 2.0 * bE
    pars[:, P_IGT] = 1.0 / gt
    pars[:, P_OFFGT] = offset / gt
    pars[:, P_CLIP] = 10.0 / tau
    pars[:, P_KT] = 0.5 * kpos
    pars[:, P_KTB] = -0.5 * kpos * NOISE_THRESH
    pars[:, P_SQL2] = sql2
    pars[:, P_NSQL2] = -sql2
    pars[:, P_NS2] = -0.5 / (sigma_r * sigma_r)
    pars2 = np.repeat(pars, 2, axis=0)  # per half-image

    in_maps = []
    per_core = HALVES // N_CORES
    for c in range(N_CORES):
        sl = slice(c * per_core, (c + 1) * per_core)
        in_maps.append({
            "xpad": np.ascontiguousarray(planar[sl]),
            "pp": np.ascontiguousarray(pars2[sl]),
        })
    return in_maps


def _host_post(results, images, params, chunks):
    cr = ROWS_PER_HALF // chunks
    il = cr * PC  # interior flat length per plane per chunk
    outs = [np.asarray(r["out"], np.float32) for r in results]
    full = np.concatenate(outs, axis=0)  # [1024, chunks, 3, il]
    full = full.reshape(HALVES, chunks, C, cr, PC)[:, :, :, :, 1:65]
    # -> [1024, 32, 64, 3]
    full = full.transpose(0, 1, 3, 4, 2).reshape(NIMG, H, W, C)

    sk = np.concatenate([np.asarray(r["accs"], np.float64)
                         for r in results], axis=0)  # [1024, 2*chunks]
    a_half = sk[:, 0:chunks].sum(axis=1)
    n_half = sk[:, chunks:2 * chunks].sum(axis=1)
    a_img = a_half[0::2] + a_half[1::2]
    n_img = n_half[0::2] + n_half[1::2]
    tau = np.clip(np.asarray(params, np.float32).reshape(NIMG, 7)[:, 4],
                  0.5, 5.0)
    skip = (a_img < MEAN_N * SKIP_THRESH) | (n_img < MEAN_N * SKIP_THRESH / tau)
    if skip.any():
        x = np.asarray(images, np.float32).reshape(NIMG, H, W, C)
        full[skip] = np.clip(x[skip], 1e-5, 1.0)
    return full.reshape(B, PP, H, W, C)


# --------------------------------------------------------------------------
# device program
# --------------------------------------------------------------------------

def build_program(cfg=None):
    import concourse.tile as tile
    from concourse import bacc, mybir
    from contextlib import ExitStack

    cfg = dict(cfg or {})
    F32 = mybir.dt.float32
    F16 = mybir.dt.float16
    ALU = mybir.AluOpType
    AF = mybir.ActivationFunctionType

    repeat = int(cfg.get("repeat", 1))
    CHUNKS = int(cfg.get("chunks", 4))
    CR = ROWS_PER_HALF // CHUNKS
    SR = CR + 2
    FL = SR * PC + BPAD      # field tile length per plane (flat + back pad)
    IL = CR * PC             # interior flat length (rows 1..CR, all 66 cols)
    IOFF = PC                # interior start offset inside a field span

    # engine knobs
    pool_pair = bool(cfg.get("pool_pair", False))
    pool_d2 = int(cfg.get("pool_d2", 0))      # taps whose d2 adds go to Pool
    pool_ptree = bool(cfg.get("pool_ptree", False))
    pool_av = bool(cfg.get("pool_av", False))
    pool_ne0 = bool(cfg.get("pool_ne0", False))
    act_sq = int(cfg.get("act_sq", 4))        # taps whose square runs on ACT
    act_t2 = bool(cfg.get("act_t2", False))
    act_nm = bool(cfg.get("act_nm", True))

    nc = bacc.Bacc("TRN2", target_bir_lowering=False, debug=False)
    xdram = nc.dram_tensor("xpad", [128, C, PLANE], F16,
                           kind="ExternalInput").ap()
    pdram = nc.dram_tensor("pp", [128, NPARAM], F32,
                           kind="ExternalInput").ap()
    odram = nc.dram_tensor("out", [128, CHUNKS, C, IL], F16,
                           kind="ExternalOutput").ap()
    adram = nc.dram_tensor("accs", [128, 2 * CHUNKS], F32,
                           kind="ExternalOutput").ap()

    with tile.TileContext(nc) as tc:
        with ExitStack() as ctx:
            pool = ctx.enter_context(tc.tile_pool(name="main", bufs=1))

            pp = pool.tile([128, NPARAM], F32, tag="pp", bufs=1)
            nc.sync.dma_start(pp[:], pdram[:])

            def par(col):
                return pp[:, col:col + 1]

            for rep in range(repeat):
              xs = pool.tile([128, C, PLANE_T], F16, tag="xs", bufs=2,
                             name=f"xs{rep}")
              nc.gpsimd.memset(xs[:, :, 0:FPAD], 0.25)
              nc.gpsimd.memset(xs[:, :, FPAD + PLANE:PLANE_T], 0.25)
              nc.sync.dma_start(xs[:, :, FPAD:FPAD + PLANE], xdram[:])
              accs = pool.tile([128, 2 * CHUNKS], F32, tag="accs", bufs=2,
                               name=f"accs{rep}")

              for ch in range(CHUNKS):
                base = FPAD + ch * CR * PC   # slab row 0, col 0 flat offset
                sfx = f"{ch}_{rep}"

                def xw(off, ln=FL):
                    # [3, ln] plane-major window of xs at slab offset `off`
                    return xs[:, :, base + off:base + off + ln]

                # ---- tap diff fields, flat [3, FL] ----
                # dW[k]  = x[k-1]  - x[k]   (west neighbor)
                # dN[k]  = x[k-66] - x[k]
                # dNW[k] = x[k-67] - x[k]
                # dNE[k] = x[k-65] - x[k]
                taps = {}
                for nm_, off, tag in [("W", -1, "dWN"), ("N", -PC, "dWN"),
                                      ("NW", -PC - 1, "dC"),
                                      ("NE", -PC + 1, "dC")]:
                    d = pool.tile([128, C, FL], F16, tag=tag,
                                  bufs=4 if tag == "dWN" else 3,
                                  name=f"d{nm_}{sfx}")
                    nc.vector.tensor_tensor(d[:], xw(off), xw(0), ALU.subtract)
                    taps[nm_] = d

                # ---- squares, channel sums -> d2, exp -> ck ----
                d2EN = pool.tile([128, 2, FL], F16, tag="d2", bufs=4,
                                 name=f"d2EN{sfx}")
                d2C = pool.tile([128, 2, FL], F16, tag="d2", bufs=4,
                                name=f"d2C{sfx}")
                for ti, (nm_, d2t, g) in enumerate(
                        [("W", d2EN, 0), ("N", d2EN, 1),
                         ("NW", d2C, 0), ("NE", d2C, 1)]):
                    dt_ = taps[nm_]
                    sq = pool.tile([128, C, FL], F16, tag="sq", bufs=3,
                                   name=f"sq{ti}_{sfx}")
                    if ti < act_sq:
                        nc.scalar.activation(sq[:], dt_[:], AF.Square,
                                             scale=par(P_S))
                    else:
                        nc.vector.tensor_tensor(sq[:], dt_[:], dt_[:],
                                                ALU.mult)
                    eng = nc.gpsimd if ti < pool_d2 else nc.vector
                    d2a = pool.tile([128, FL], F16, tag="d2a", bufs=2,
                                    name=f"d2a{ti}_{sfx}")
                    eng.tensor_tensor(d2a[:], sq[:, 0], sq[:, 1], ALU.add)
                    eng.tensor_tensor(d2t[:, g], d2a[:], sq[:, 2], ALU.add)

                esc = -1.0 if act_sq else par(P_NS2)
                ckEN = pool.tile([128, 2, FL], F16, tag="ck", bufs=4,
                                 name=f"ckEN{sfx}")
                nc.scalar.activation(ckEN[:], d2EN[:], AF.Exp,
                                     bias=par(P_LOGE), scale=esc)
                ckC = pool.tile([128, 2, FL], F16, tag="ck", bufs=4,
                                name=f"ckC{sfx}")
                nc.scalar.activation(ckC[:], d2C[:], AF.Exp,
                                     bias=par(P_LOGC), scale=esc)

                # ---- prod, dif = prod_I - prod_M, pair ----
                # M offset in flat coords = -(tap offset)
                TAPW = [("W", ckEN, 0, 1), ("N", ckEN, 1, PC),
                        ("NW", ckC, 0, PC + 1), ("NE", ckC, 1, PC - 1)]
                difs = []
                pairs = []
                for ti, (nm_, ckt, g, mo) in enumerate(TAPW):
                    dt_ = taps[nm_]
                    prod = pool.tile([128, C, FL], F16, tag="prod", bufs=4,
                                     name=f"pr{ti}_{sfx}")
                    nc.vector.tensor_tensor(
                        prod[:], dt_[:],
                        ckt[:, g].unsqueeze(1).broadcast_to([128, C, FL]),
                        ALU.mult)
                    dif = pool.tile([128, C, IL], F16, tag="dif", bufs=4,
                                    name=f"dif{ti}_{sfx}")
                    nc.vector.tensor_tensor(
                        dif[:], prod[:, :, IOFF:IOFF + IL],
                        prod[:, :, IOFF + mo:IOFF + mo + IL], ALU.subtract)
                    difs.append(dif)
                    pair = pool.tile([128, IL], F16, tag="pair", bufs=6,
                                     name=f"pai{ti}_{sfx}")
                    peng = nc.gpsimd if pool_pair else nc.vector
                    peng.tensor_tensor(
                        pair[:], ckt[:, g, IOFF:IOFF + IL],
                        ckt[:, g, IOFF + mo:IOFF + mo + IL], ALU.add)
                    pairs.append(pair)

                n01 = pool.tile([128, C, IL], F16, tag="nacc", bufs=3,
                                name=f"n01{sfx}")
                nc.vector.tensor_tensor(n01[:], difs[0][:], difs[1][:],
                                        ALU.add)
                n23 = pool.tile([128, C, IL], F16, tag="nacc", bufs=3,
                                name=f"n23{sfx}")
                nc.vector.tensor_tensor(n23[:], difs[2][:], difs[3][:],
                                        ALU.add)
                nacc = pool.tile([128, C, IL], F16, tag="nacc", bufs=3,
                                 name=f"nacc{sfx}")
                nc.vector.tensor_tensor(nacc[:], n01[:], n23[:], ALU.add)

                teng = nc.gpsimd if pool_ptree else nc.vector
                p01 = pool.tile([128, IL], F16, tag="ptree", bufs=3,
                                name=f"p01{sfx}")
                teng.tensor_tensor(p01[:], pairs[0][:], pairs[1][:], ALU.add)
                p23 = pool.tile([128, IL], F16, tag="ptree", bufs=3,
                                name=f"p23{sfx}")
                teng.tensor_tensor(p23[:], pairs[2][:], pairs[3][:], ALU.add)
                q = pool.tile([128, IL], F16, tag="ptree", bufs=3,
                              name=f"q{sfx}")
                teng.tensor_tensor(q[:], p01[:], p23[:], ALU.add)
                sacc = pool.tile([128, IL], F32, tag="sacc", bufs=2,
                                 name=f"sacc{sfx}")
                nc.vector.tensor_scalar(sacc[:], q[:], par(P_WSC), None,
                                        ALU.add)
                rS = pool.tile([128, IL], F32, tag="rS", bufs=2,
                               name=f"rS{sfx}")
                nc.vector.reciprocal_approx_fast(rS[:], sacc[:])
                rs16 = pool.tile([128, IL], F16, tag="rs16", bufs=2,
                                 name=f"rs16{sfx}")
                nc.vector.tensor_scalar(rs16[:], rS[:], 1.0, None, ALU.mult)
                tT = pool.tile([128, C, IL], F16, tag="tT", bufs=2,
                               name=f"tT{sfx}")
                nc.vector.tensor_tensor(
                    tT[:], nacc[:],
                    rs16[:].unsqueeze(1).broadcast_to([128, C, IL]), ALU.mult)

                # ---- gaussian detail from dW/dN ----
                # H[k] = dW[k+2] - dW[k+1]  (= -Lx at flat pos k+1)
                dW, dN = taps["W"], taps["N"]
                Hf = pool.tile([128, C, FL], F16, tag="H", bufs=2,
                               name=f"H{sfx}")
                nc.vector.tensor_tensor(
                    Hf[:, :, 0:FL - 1], dW[:, :, 1:FL], dW[:, :, 0:FL - 1],
                    ALU.subtract)
                V = pool.tile([128, C, IL], F16, tag="s1", bufs=5,
                              name=f"V{sfx}")
                nc.vector.tensor_tensor(
                    V[:], dN[:, :, IOFF + PC:IOFF + PC + IL],
                    dN[:, :, IOFF:IOFF + IL], ALU.subtract)
                av = pool.tile([128, C, IL], F16, tag="s1", bufs=5,
                               name=f"av{sfx}")
                (nc.gpsimd if pool_av else nc.vector).tensor_tensor(
                    av[:], Hf[:, :, IOFF - PC:IOFF - PC + IL],
                    Hf[:, :, IOFF + PC:IOFF + PC + IL], ALU.add)
                c1 = pool.tile([128, C, IL], F16, tag="s1", bufs=5,
                               name=f"c1{sfx}")
                nc.vector.tensor_scalar(c1[:], Hf[:, :, IOFF:IOFF + IL],
                                        par(P_1M2BE), None, ALU.mult)
                c2 = pool.tile([128, C, IL], F16, tag="s1", bufs=5,
                               name=f"c2{sfx}")
                nc.vector.tensor_scalar(c2[:], av[:], par(P_BE), None,
                                        ALU.mult)
                c3 = pool.tile([128, C, IL], F16, tag="s1", bufs=5,
                               name=f"c3{sfx}")
                nc.vector.tensor_tensor(c3[:], V[:], c2[:], ALU.add)
                inner = pool.tile([128, C, IL], F16, tag="inner", bufs=2,
                                  name=f"inner{sfx}")
                nc.vector.tensor_tensor(inner[:], c1[:], c3[:], ALU.add)
                # zero the two pad columns so accumulators stay exact
                iv = inner[:].rearrange("p c (r s) -> p c r s", s=PC)
                nc.gpsimd.memset(iv[:, :, :, 0:1], 0.0)
                nc.gpsimd.memset(iv[:, :, :, 65:66], 0.0)

                # ---- noise / masks ----
                adet = pool.tile([128, C, IL], F16, tag="s1", bufs=5,
                                 name=f"adet{sfx}")
                nc.scalar.activation(adet[:], inner[:], AF.Abs,
                                     scale=par(P_BE),
                                     accum_out=accs[:, ch:ch + 1])
                th = pool.tile([128, C, IL], F16, tag="s1", bufs=5,
                               name=f"th{sfx}")
                nc.scalar.activation(th[:], adet[:], AF.Tanh,
                                     bias=par(P_KTB), scale=par(P_KT))
                th1 = pool.tile([128, C, IL], F16, tag="th1", bufs=2,
                                name=f"th1{sfx}")
                nc.vector.tensor_scalar(th1[:], th[:], 0.5, 0.5,
                                        ALU.mult, ALU.add)
                d1 = pool.tile([128, C, IL], F32, tag="d32", bufs=2,
                               name=f"d1{sfx}")
                nc.scalar.activation(d1[:], xw(IOFF, IL), AF.Identity,
                                     scale=par(P_IGT), bias=par(P_OFFGT))
                r1 = pool.tile([128, C, IL], F32, tag="d32", bufs=2,
                               name=f"r1{sfx}")
                nc.vector.reciprocal_approx_fast(
                    r1[:].rearrange("p a b -> p (a b)"),
                    d1[:].rearrange("p a b -> p (a b)"))
                ne0 = pool.tile([128, C, IL], F16, tag="s1", bufs=5,
                                name=f"ne0{sfx}")
                (nc.gpsimd if pool_ne0 else nc.vector).tensor_tensor(
                    ne0[:], adet[:], r1[:], ALU.mult)
                neq = pool.tile([128, C, IL], F16, tag="s1", bufs=5,
                                name=f"neq{sfx}")
                nc.vector.tensor_scalar(
                    neq[:], ne0[:], par(P_CLIP), None, ALU.min, ALU.add,
                    accum_out=accs[:, CHUNKS + ch:CHUNKS + ch + 1])
                sqn = pool.tile([128, C, IL], F16, tag="s1", bufs=5,
                                name=f"sqn{sfx}")
                nc.scalar.activation(sqn[:], neq[:], AF.Square)
                ee = pool.tile([128, C, IL], F16, tag="s1", bufs=5,
                               name=f"ee{sfx}")
                nc.scalar.activation(ee[:], sqn[:], AF.Exp, scale=-1.0)
                t2 = pool.tile([128, C, IL], F16, tag="s1", bufs=5,
                               name=f"t2{sfx}")
                if act_t2:
                    nc.scalar.activation(t2[:], ee[:], AF.Identity,
                                         scale=par(P_NSQL2), bias=par(P_SQL2))
                else:
                    nc.vector.tensor_scalar(t2[:], ee[:], par(P_NSQL2),
                                            par(P_SQL2), ALU.mult, ALU.add)
                nm = pool.tile([128, C, IL], F16, tag="s1", bufs=5,
                               name=f"nm{sfx}")
                if act_nm:
                    nc.scalar.activation(nm[:], t2[:], AF.Square)
                else:
                    nc.vector.tensor_tensor(nm[:], t2[:], t2[:], ALU.mult)
                s3 = pool.tile([128, C, IL], F16, tag="s1", bufs=5,
                               name=f"s3{sfx}")
                nc.vector.tensor_tensor(s3[:], th1[:], nm[:], ALU.mult)
                sharp = pool.tile([128, C, IL], F16, tag="s1", bufs=5,
                                  name=f"sh{sfx}")
                nc.vector.tensor_tensor(sharp[:], s3[:], inner[:], ALU.mult)

                # ---- combine + clip + store ----
                t3 = pool.tile([128, C, IL], F16, tag="s1", bufs=5,
                               name=f"t3{sfx}")
                nc.vector.tensor_tensor(t3[:], tT[:], sharp[:], ALU.add)
                o3a = pool.tile([128, C, IL], F16, tag="s1", bufs=5,
                                name=f"o3a{sfx}")
                nc.vector.tensor_tensor(o3a[:], xw(IOFF, IL), t3[:], ALU.add)
                o3 = pool.tile([128, C, IL], F16, tag="o3", bufs=2,
                               name=f"o3{sfx}")
                nc.vector.tensor_scalar(o3[:], o3a[:], 1e-5, 1.0,
                                        ALU.max, ALU.min)
                nc.sync.dma_start(odram[:, ch], o3[:])

              nc.sync.dma_start(adram[:], accs[:])

    nc.compile()
    return nc


def _get_program(cfg=None):
    key = tuple(sorted((cfg or {}).items()))
    if key not in _CACHE:
        _CACHE[key] = build_program(cfg)
    return _CACHE[key]


# --------------------------------------------------------------------------
# entry point
# --------------------------------------------------------------------------

DEFAULT_CFG = {}


def kernel(images, params, k):
    from concourse.bass_utils import run_bass_kernel_spmd

    cfg = dict(DEFAULT_CFG)
    nc = _get_program(cfg)
    in_maps = _host_prep(np.asarray(images), np.asarray(params), np.asarray(k))
    res = run_bass_kernel_spmd(nc, in_maps, list(range(N_CORES)))
    chunks = int(cfg.get("chunks", 4))
    return _host_post(res.results, images, params, chunks).astype(np.float32)


# revision 31
# speedup vs baseline: 3.7049x; 3.7049x over previous
"""Trainium2 Bass kernel for nn_DenoisingSharpening (v3, wrap-flat layout).

Contract: kernel(**inputs) takes the FULL unsharded inputs
(images [8,64,64,64,3] f32, params [8,64,7] f32, k [] f32) and returns
the FULL output [8,64,64,64,3] f32.

Strategy
--------
Data-parallel over N = B*P = 512 images; 64 images per NeuronCore, one
half-image (32 rows) per SBUF partition -> 128 partitions x 8 cores.

Real TRN2 engines pay a large per-segment tax on multi-row strided
access patterns, so every op here works on FLAT per-channel-plane spans
("wrap" layout): a +-1 row/col stencil shift is just a +-1/+-66 flat
offset.  Flat-offset neighbor arithmetic is exact for interior columns;
the two pad columns per row carry wrap garbage which never feeds an
interior lane (checked per tap) and is stripped on the host.  The two
pad columns of `inner` are zeroed so the skip-mean accumulators stay
exact.

Per chunk of CR rows: 4 bilateral tap fields (W, NW, N, NE) as flat
diffs of the planar f16 input; ck = exp(-(s*d)^2 sum + log w) with
squares on ACT and the exp batched per weight class; numerator via the
symmetric-pair trick dif = prod_I - prod_M; denominator from flat ck
adds; bf - x = nacc/sacc via reciprocal_approx_fast; separable gaussian
detail rebuilt from the same dW/dN fields; noise chain on ACT (single
table set: Square/Exp/Abs/Tanh/Identity/Copy); skip decision on host
from shipped per-chunk sums.
"""

import numpy as np

N_CORES = 8
B, PP, H, W, C = 8, 64, 64, 64, 3
NIMG = B * PP          # 512
HALVES = 2 * NIMG      # 1024 half-images, 128 per core
PR, PC = 34, 66        # padded half-image rows/cols
ROWS_PER_HALF = 32
PLANE = PR * PC        # 2244 flat elems per channel plane
FPAD = 68              # front pad per plane (worst back-read -67)
BPAD = 4               # back pad (H reads +2 past slab end)
PLANE_T = FPAD + PLANE + BPAD   # 2316 per-plane tile pitch

NOISE_THRESH = 0.002
SKIP_THRESH = 1e-4
MEAN_N = float(C * H * W)

# params columns
(P_S, P_LOGE, P_LOGC, P_WSC, P_BE, P_1M2BE, P_IGT, P_OFFGT, P_CLIP,
 P_KT, P_KTB, P_SQL2, P_NSQL2, P_NS2, P_HLB, P_M1) = range(16)
NPARAM = 16

_CACHE = {}


# --------------------------------------------------------------------------
# host-side preprocessing
# --------------------------------------------------------------------------

def _host_prep(images, params, k):
    x = np.ascontiguousarray(images, dtype=np.float32).reshape(NIMG, H, W, C)
    xp = np.pad(x, ((0, 0), (1, 1), (1, 1), (0, 0)), mode="reflect")
    halves = np.stack([xp[:, 0:PR], xp[:, ROWS_PER_HALF:ROWS_PER_HALF + PR]],
                      axis=1).reshape(HALVES, PR, PC, C)
    planar = np.ascontiguousarray(
        halves.transpose(0, 3, 1, 2), dtype=np.float16).reshape(
            HALVES, C, PLANE)

    p = np.asarray(params, dtype=np.float32).reshape(NIMG, 7)
    sigma_r = np.clip(p[:, 1], 0.01, 1.0)
    sigma_s = np.clip(p[:, 0], 0.2, 5.0)
    sigma_f = np.clip(p[:, 2], 0.2, 3.0)
    lam = np.clip(p[:, 3], 0.1, 2.0)
    tau = np.clip(p[:, 4], 0.5, 5.0)
    gain = np.clip(p[:, 5], 0.2, 2.0)
    offset = np.clip(p[:, 6], 0.01, 1.0)

    def gauss1d(sig):
        g = np.exp(-0.5 * (np.array([-1.0, 0.0, 1.0], np.float32)[None, :]
                           / sig[:, None]) ** 2)
        return g / g.sum(axis=1, keepdims=True)

    gs = gauss1d(sigma_s)
    gf = gauss1d(sigma_f)
    aE, aC = gs[:, 0], gs[:, 1]
    bE = gf[:, 0]

    kpos = max(abs(float(np.asarray(k))), 1.0)
    gt = gain / tau
    sql2 = np.sqrt(lam * bE)

    pars = np.zeros((NIMG, NPARAM), np.float32)
    pars[:, P_S] = np.sqrt(0.5) / sigma_r
    pars[:, P_LOGE] = np.log(aE * aC)
    pars[:, P_LOGC] = np.log(aE * aE)
    pars[:, P_WSC] = aC * aC
    pars[:, P_BE] = bE
    pars[:, P_1M2BE] = 1.0 - 2.0 * bE
    pars[:, P_IGT] = 1.0 / gt
    pars[:, P_OFFGT] = offset / gt
    pars[:, P_CLIP] = 10.0 / tau
    pars[:, P_KT] = 0.5 * kpos
    pars[:, P_KTB] = -0.5 * kpos * NOISE_THRESH
    pars[:, P_SQL2] = sql2
    pars[:, P_NSQL2] = -sql2
    pars[:, P_NS2] = -0.5 / (sigma_r * sigma_r)
    pars[:, P_HLB] = 0.5 * lam * bE
    pars[:, P_M1] = -1.0
    pars2 = np.repeat(pars, 2, axis=0)  # per half-image

    in_maps = []
    per_core = HALVES // N_CORES
    for c in range(N_CORES):
        sl = slice(c * per_core, (c + 1) * per_core)
        in_maps.append({
            "xpad": np.ascontiguousarray(planar[sl]),
            "pp": np.ascontiguousarray(pars2[sl]),
        })
    return in_maps


def _host_post(results, images, params, chunks):
    cr = ROWS_PER_HALF // chunks
    il = cr * PC  # interior flat length per plane per chunk
    outs = [np.asarray(r["out"], np.float32) for r in results]
    full = np.concatenate(outs, axis=0)  # [1024, chunks, 3, il]
    full = full.reshape(HALVES, chunks, C, cr, PC)[:, :, :, :, 1:65]
    # -> [1024, 32, 64, 3]
    full = full.transpose(0, 1, 3, 4, 2).reshape(NIMG, H, W, C)

    sk = np.concatenate([np.asarray(r["accs"], np.float64)
                         for r in results], axis=0)  # [1024, 2*chunks]
    a_half = sk[:, 0:chunks].sum(axis=1)
    n_half = sk[:, chunks:2 * chunks].sum(axis=1)
    a_img = a_half[0::2] + a_half[1::2]
    n_img = n_half[0::2] + n_half[1::2]
    tau = np.clip(np.asarray(params, np.float32).reshape(NIMG, 7)[:, 4],
                  0.5, 5.0)
    skip = (a_img < MEAN_N * SKIP_THRESH) | (n_img < MEAN_N * SKIP_THRESH / tau)
    if skip.any():
        x = np.asarray(images, np.float32).reshape(NIMG, H, W, C)
        full[skip] = np.clip(x[skip], 1e-5, 1.0)
    return full.reshape(B, PP, H, W, C)


# --------------------------------------------------------------------------
# device program
# --------------------------------------------------------------------------

def build_program(cfg=None):
    import concourse.tile as tile
    from concourse import bacc, mybir
    from contextlib import ExitStack

    cfg = dict(cfg or {})
    F32 = mybir.dt.float32
    F16 = mybir.dt.float16
    ALU = mybir.AluOpType
    AF = mybir.ActivationFunctionType

    repeat = int(cfg.get("repeat", 1))
    CHUNKS = int(cfg.get("chunks", 4))
    CR = ROWS_PER_HALF // CHUNKS
    SR = CR + 2
    FL = SR * PC + BPAD      # field tile length per plane (flat + back pad)
    IL = CR * PC             # interior flat length (rows 1..CR, all 66 cols)
    IOFF = PC                # interior start offset inside a field span

    # engine knobs
    pool_pair = bool(cfg.get("pool_pair", False))
    pool_d2 = int(cfg.get("pool_d2", 0))      # taps whose d2 adds go to Pool
    pool_ptree = bool(cfg.get("pool_ptree", False))
    pool_av = bool(cfg.get("pool_av", False))
    pool_ne0 = bool(cfg.get("pool_ne0", False))
    act_sq = int(cfg.get("act_sq", 4))        # taps whose square runs on ACT
    act_t2 = bool(cfg.get("act_t2", False))
    act_nm = bool(cfg.get("act_nm", True))

    nc = bacc.Bacc("TRN2", target_bir_lowering=False, debug=False)
    xdram = nc.dram_tensor("xpad", [128, C, PLANE], F16,
                           kind="ExternalInput").ap()
    pdram = nc.dram_tensor("pp", [128, NPARAM], F32,
                           kind="ExternalInput").ap()
    odram = nc.dram_tensor("out", [128, CHUNKS, C, IL], F16,
                           kind="ExternalOutput").ap()
    adram = nc.dram_tensor("accs", [128, 2 * CHUNKS], F32,
                           kind="ExternalOutput").ap()

    with tile.TileContext(nc) as tc:
        with ExitStack() as ctx:
            pool = ctx.enter_context(tc.tile_pool(name="main", bufs=1))

            pp = pool.tile([128, NPARAM], F32, tag="pp", bufs=1)
            nc.sync.dma_start(pp[:], pdram[:])

            def par(col):
                return pp[:, col:col + 1]

            for rep in range(repeat):
              xs = pool.tile([128, C, PLANE_T], F16, tag="xs", bufs=2,
                             name=f"xs{rep}")
              nc.gpsimd.memset(xs[:, :, 0:FPAD], 0.25)
              nc.gpsimd.memset(xs[:, :, FPAD + PLANE:PLANE_T], 0.25)
              nc.sync.dma_start(xs[:, :, FPAD:FPAD + PLANE], xdram[:])
              accs = pool.tile([128, 2 * CHUNKS], F32, tag="accs", bufs=2,
                               name=f"accs{rep}")

              for ch in range(CHUNKS):
                base = FPAD + ch * CR * PC   # slab row 0, col 0 flat offset
                sfx = f"{ch}_{rep}"

                def xw(off, ln=FL):
                    # [3, ln] plane-major window of xs at slab offset `off`
                    return xs[:, :, base + off:base + off + ln]

                # ---- tap diff fields, flat [3, FL] ----
                # dW[k]  = x[k-1]  - x[k]   (west neighbor)
                # dN[k]  = x[k-66] - x[k]
                # dNW[k] = x[k-67] - x[k]
                # dNE[k] = x[k-65] - x[k]
                taps = {}
                for nm_, off, tag in [("W", -1, "dWN"), ("N", -PC, "dWN"),
                                      ("NW", -PC - 1, "dC"),
                                      ("NE", -PC + 1, "dC")]:
                    d = pool.tile([128, C, FL], F16, tag=tag,
                                  bufs=4 if tag == "dWN" else 3,
                                  name=f"d{nm_}{sfx}")
                    nc.vector.tensor_tensor(d[:], xw(off), xw(0), ALU.subtract)
                    taps[nm_] = d

                # ---- squares, channel sums -> d2, exp -> ck ----
                d2EN = pool.tile([128, 2, FL], F16, tag="d2", bufs=4,
                                 name=f"d2EN{sfx}")
                d2C = pool.tile([128, 2, FL], F16, tag="d2", bufs=4,
                                name=f"d2C{sfx}")
                for ti, (nm_, d2t, g) in enumerate(
                        [("W", d2EN, 0), ("N", d2EN, 1),
                         ("NW", d2C, 0), ("NE", d2C, 1)]):
                    dt_ = taps[nm_]
                    sq = pool.tile([128, C, FL], F16, tag="sq", bufs=3,
                                   name=f"sq{ti}_{sfx}")
                    if ti < act_sq:
                        nc.scalar.activation(sq[:], dt_[:], AF.Square,
                                             scale=par(P_S))
                    else:
                        nc.vector.tensor_tensor(sq[:], dt_[:], dt_[:],
                                                ALU.mult)
                    eng = nc.gpsimd if ti < pool_d2 else nc.vector
                    d2a = pool.tile([128, FL], F16, tag="d2a", bufs=2,
                                    name=f"d2a{ti}_{sfx}")
                    eng.tensor_tensor(d2a[:], sq[:, 0], sq[:, 1], ALU.add)
                    eng.tensor_tensor(d2t[:, g], d2a[:], sq[:, 2], ALU.add)

                esc = -1.0 if act_sq else par(P_NS2)
                ckEN = pool.tile([128, 2, FL], F16, tag="ck", bufs=4,
                                 name=f"ckEN{sfx}")
                nc.scalar.activation(ckEN[:], d2EN[:], AF.Exp,
                                     bias=par(P_LOGE), scale=esc)
                ckC = pool.tile([128, 2, FL], F16, tag="ck", bufs=4,
                                name=f"ckC{sfx}")
                nc.scalar.activation(ckC[:], d2C[:], AF.Exp,
                                     bias=par(P_LOGC), scale=esc)

                # ---- prod, dif = prod_I - prod_M, pair ----
                # M offset in flat coords = -(tap offset)
                TAPW = [("W", ckEN, 0, 1), ("N", ckEN, 1, PC),
                        ("NW", ckC, 0, PC + 1), ("NE", ckC, 1, PC - 1)]
                difs = []
                pairs = []
                for ti, (nm_, ckt, g, mo) in enumerate(TAPW):
                    dt_ = taps[nm_]
                    prod = pool.tile([128, C, FL], F16, tag="prod", bufs=4,
                                     name=f"pr{ti}_{sfx}")
                    nc.vector.tensor_tensor(
                        prod[:], dt_[:],
                        ckt[:, g].unsqueeze(1).broadcast_to([128, C, FL]),
                        ALU.mult)
                    dif = pool.tile([128, C, IL], F16, tag="dif", bufs=4,
                                    name=f"dif{ti}_{sfx}")
                    nc.vector.tensor_tensor(
                        dif[:], prod[:, :, IOFF:IOFF + IL],
                        prod[:, :, IOFF + mo:IOFF + mo + IL], ALU.subtract)
                    difs.append(dif)
                    pair = pool.tile([128, IL], F16, tag="pair", bufs=6,
                                     name=f"pai{ti}_{sfx}")
                    peng = nc.gpsimd if pool_pair else nc.vector
                    peng.tensor_tensor(
                        pair[:], ckt[:, g, IOFF:IOFF + IL],
                        ckt[:, g, IOFF + mo:IOFF + mo + IL], ALU.add)
                    pairs.append(pair)

                n01 = pool.tile([128, C, IL], F16, tag="nacc", bufs=3,
                                name=f"n01{sfx}")
                nc.vector.tensor_tensor(n01[:], difs[0][:], difs[1][:],
                                        ALU.add)
                n23 = pool.tile([128, C, IL], F16, tag="nacc", bufs=3,
                                name=f"n23{sfx}")
                nc.vector.tensor_tensor(n23[:], difs[2][:], difs[3][:],
                                        ALU.add)
                nacc = pool.tile([128, C, IL], F16, tag="nacc", bufs=3,
                                 name=f"nacc{sfx}")
                nc.vector.tensor_tensor(nacc[:], n01[:], n23[:], ALU.add)

                teng = nc.gpsimd if pool_ptree else nc.vector
                p01 = pool.tile([128, IL], F16, tag="ptree", bufs=3,
                                name=f"p01{sfx}")
                teng.tensor_tensor(p01[:], pairs[0][:], pairs[1][:], ALU.add)
                p23 = pool.tile([128, IL], F16, tag="ptree", bufs=3,
                                name=f"p23{sfx}")
                teng.tensor_tensor(p23[:], pairs[2][:], pairs[3][:], ALU.add)
                q = pool.tile([128, IL], F16, tag="ptree", bufs=3,
                              name=f"q{sfx}")
                teng.tensor_tensor(q[:], p01[:], p23[:], ALU.add)
                sacc = pool.tile([128, IL], F32, tag="sacc", bufs=2,
                                 name=f"sacc{sfx}")
                nc.vector.tensor_scalar(sacc[:], q[:], par(P_WSC), None,
                                        ALU.add)
                rS = pool.tile([128, IL], F32, tag="rS", bufs=2,
                               name=f"rS{sfx}")
                nc.vector.reciprocal_approx_fast(rS[:], sacc[:])
                tT = pool.tile([128, C, IL], F16, tag="tT", bufs=2,
                               name=f"tT{sfx}")
                nc.vector.tensor_tensor(
                    tT[:], nacc[:],
                    rS[:].unsqueeze(1).broadcast_to([128, C, IL]), ALU.mult)

                # ---- gaussian detail from dW/dN ----
                # H[k] = dW[k+2] - dW[k+1]  (= -Lx at flat pos k+1)
                dW, dN = taps["W"], taps["N"]
                Hf = pool.tile([128, C, FL], F16, tag="H", bufs=2,
                               name=f"H{sfx}")
                nc.vector.tensor_tensor(
                    Hf[:, :, 0:FL - 1], dW[:, :, 1:FL], dW[:, :, 0:FL - 1],
                    ALU.subtract)
                V = pool.tile([128, C, IL], F16, tag="s1", bufs=5,
                              name=f"V{sfx}")
                nc.vector.tensor_tensor(
                    V[:], dN[:, :, IOFF + PC:IOFF + PC + IL],
                    dN[:, :, IOFF:IOFF + IL], ALU.subtract)
                av = pool.tile([128, C, IL], F16, tag="s1", bufs=5,
                               name=f"av{sfx}")
                (nc.gpsimd if pool_av else nc.vector).tensor_tensor(
                    av[:], Hf[:, :, IOFF - PC:IOFF - PC + IL],
                    Hf[:, :, IOFF + PC:IOFF + PC + IL], ALU.add)
                c1 = pool.tile([128, C, IL], F16, tag="s1", bufs=5,
                               name=f"c1{sfx}")
                nc.vector.tensor_scalar(c1[:], Hf[:, :, IOFF:IOFF + IL],
                                        par(P_1M2BE), None, ALU.mult)
                c2 = pool.tile([128, C, IL], F16, tag="s1", bufs=5,
                               name=f"c2{sfx}")
                nc.vector.tensor_scalar(c2[:], av[:], par(P_BE), None,
                                        ALU.mult)
                c3 = pool.tile([128, C, IL], F16, tag="s1", bufs=5,
                               name=f"c3{sfx}")
                nc.vector.tensor_tensor(c3[:], V[:], c2[:], ALU.add)
                inner = pool.tile([128, C, IL], F16, tag="inner", bufs=2,
                                  name=f"inner{sfx}")
                nc.vector.tensor_tensor(inner[:], c1[:], c3[:], ALU.add)
                # zero the two pad columns so accumulators stay exact
                iv = inner[:].rearrange("p c (r s) -> p c r s", s=PC)
                nc.gpsimd.memset(iv[:, :, :, 0:1], 0.0)
                nc.gpsimd.memset(iv[:, :, :, 65:66], 0.0)

                # ---- noise / masks ----
                adet = pool.tile([128, C, IL], F16, tag="s1", bufs=5,
                                 name=f"adet{sfx}")
                nc.scalar.activation(adet[:], inner[:], AF.Abs,
                                     scale=par(P_BE),
                                     accum_out=accs[:, ch:ch + 1])
                th = pool.tile([128, C, IL], F16, tag="s1", bufs=5,
                               name=f"th{sfx}")
                nc.scalar.activation(th[:], adet[:], AF.Tanh,
                                     bias=par(P_KTB), scale=par(P_KT))
                th1 = pool.tile([128, C, IL], F16, tag="th1", bufs=2,
                                name=f"th1{sfx}")
                nc.vector.tensor_scalar(th1[:], th[:], par(P_HLB),
                                        par(P_HLB), ALU.mult, ALU.add)
                d1 = pool.tile([128, C, IL], F32, tag="d32", bufs=2,
                               name=f"d1{sfx}")
                nc.scalar.activation(d1[:], xw(IOFF, IL), AF.Identity,
                                     scale=par(P_IGT), bias=par(P_OFFGT))
                r1 = pool.tile([128, C, IL], F32, tag="d32", bufs=2,
                               name=f"r1{sfx}")
                nc.vector.reciprocal_approx_fast(
                    r1[:].rearrange("p a b -> p (a b)"),
                    d1[:].rearrange("p a b -> p (a b)"))
                ne0 = pool.tile([128, C, IL], F16, tag="s1", bufs=5,
                                name=f"ne0{sfx}")
                (nc.gpsimd if pool_ne0 else nc.vector).tensor_tensor(
                    ne0[:], adet[:], r1[:], ALU.mult)
                neq = pool.tile([128, C, IL], F16, tag="s1", bufs=5,
                                name=f"neq{sfx}")
                nc.vector.tensor_scalar(
                    neq[:], ne0[:], par(P_CLIP), None, ALU.min, ALU.add,
                    accum_out=accs[:, CHUNKS + ch:CHUNKS + ch + 1])
                sqn = pool.tile([128, C, IL], F16, tag="s1", bufs=5,
                                name=f"sqn{sfx}")
                nc.scalar.activation(sqn[:], neq[:], AF.Square)
                ee = pool.tile([128, C, IL], F16, tag="s1", bufs=5,
                               name=f"ee{sfx}")
                nc.scalar.activation(ee[:], sqn[:], AF.Exp, scale=-1.0)
                nm = pool.tile([128, C, IL], F16, tag="s1", bufs=5,
                               name=f"nm{sfx}")
                nc.scalar.activation(nm[:], ee[:], AF.Square, scale=1.0,
                                     bias=par(P_M1))
                s3 = pool.tile([128, C, IL], F16, tag="s1", bufs=5,
                               name=f"s3{sfx}")
                nc.vector.tensor_tensor(s3[:], th1[:], nm[:], ALU.mult)
                sharp = pool.tile([128, C, IL], F16, tag="s1", bufs=5,
                                  name=f"sh{sfx}")
                nc.vector.tensor_tensor(sharp[:], s3[:], inner[:], ALU.mult)

                # ---- combine + clip + store ----
                t3 = pool.tile([128, C, IL], F16, tag="s1", bufs=5,
                               name=f"t3{sfx}")
                nc.vector.tensor_tensor(t3[:], tT[:], sharp[:], ALU.add)
                o3a = pool.tile([128, C, IL], F16, tag="s1", bufs=5,
                                name=f"o3a{sfx}")
                nc.vector.tensor_tensor(o3a[:], xw(IOFF, IL), t3[:], ALU.add)
                o3 = pool.tile([128, C, IL], F16, tag="o3", bufs=2,
                               name=f"o3{sfx}")
                nc.vector.tensor_scalar(o3[:], o3a[:], 1e-5, 1.0,
                                        ALU.max, ALU.min)
                nc.sync.dma_start(odram[:, ch], o3[:])

              nc.sync.dma_start(adram[:], accs[:])

    nc.compile()
    return nc


def _get_program(cfg=None):
    key = tuple(sorted((cfg or {}).items()))
    if key not in _CACHE:
        _CACHE[key] = build_program(cfg)
    return _CACHE[key]


# --------------------------------------------------------------------------
# entry point
# --------------------------------------------------------------------------

DEFAULT_CFG = {}


def kernel(images, params, k):
    from concourse.bass_utils import run_bass_kernel_spmd

    cfg = dict(DEFAULT_CFG)
    nc = _get_program(cfg)
    in_maps = _host_prep(np.asarray(images), np.asarray(params), np.asarray(k))
    res = run_bass_kernel_spmd(nc, in_maps, list(range(N_CORES)))
    chunks = int(cfg.get("chunks", 4))
    return _host_post(res.results, images, params, chunks).astype(np.float32)


# revision 32
# speedup vs baseline: 3.9771x; 1.0735x over previous
"""Trainium2 Bass kernel for nn_DenoisingSharpening (v3, wrap-flat layout).

Contract: kernel(**inputs) takes the FULL unsharded inputs
(images [8,64,64,64,3] f32, params [8,64,7] f32, k [] f32) and returns
the FULL output [8,64,64,64,3] f32.

Strategy
--------
Data-parallel over N = B*P = 512 images; 64 images per NeuronCore, one
half-image (32 rows) per SBUF partition -> 128 partitions x 8 cores.

Real TRN2 engines pay a large per-segment tax on multi-row strided
access patterns, so every op here works on FLAT per-channel-plane spans
("wrap" layout): a +-1 row/col stencil shift is just a +-1/+-66 flat
offset.  Flat-offset neighbor arithmetic is exact for interior columns;
the two pad columns per row carry wrap garbage which never feeds an
interior lane (checked per tap) and is stripped on the host.  The two
pad columns of `inner` are zeroed so the skip-mean accumulators stay
exact.

Per chunk of CR rows: 4 bilateral tap fields (W, NW, N, NE) as flat
diffs of the planar f16 input; ck = exp(-(s*d)^2 sum + log w) with
squares on ACT and the exp batched per weight class; numerator via the
symmetric-pair trick dif = prod_I - prod_M; denominator from flat ck
adds; bf - x = nacc/sacc via reciprocal_approx_fast; separable gaussian
detail rebuilt from the same dW/dN fields; noise chain on ACT (single
table set: Square/Exp/Abs/Tanh/Identity/Copy); skip decision on host
from shipped per-chunk sums.
"""

import numpy as np

N_CORES = 8
B, PP, H, W, C = 8, 64, 64, 64, 3
NIMG = B * PP          # 512
HALVES = 2 * NIMG      # 1024 half-images, 128 per core
PR, PC = 34, 66        # padded half-image rows/cols
ROWS_PER_HALF = 32
PLANE = PR * PC        # 2244 flat elems per channel plane
FPAD = 68              # front pad per plane (worst back-read -67)
BPAD = 4               # back pad (H reads +2 past slab end)
PLANE_T = FPAD + PLANE + BPAD   # 2316 per-plane tile pitch

NOISE_THRESH = 0.002
SKIP_THRESH = 1e-4
MEAN_N = float(C * H * W)

# params columns
(P_S, P_LOGE, P_LOGC, P_WSC, P_BE, P_1M2BE, P_IGT, P_OFFGT, P_CLIP,
 P_KT, P_KTB, P_SQL2, P_NSQL2, P_NS2, P_HLB, P_M1) = range(16)
NPARAM = 16

_CACHE = {}


# --------------------------------------------------------------------------
# host-side preprocessing
# --------------------------------------------------------------------------

def _host_prep(images, params, k):
    x = np.ascontiguousarray(images, dtype=np.float32).reshape(NIMG, H, W, C)
    xp = np.pad(x, ((0, 0), (1, 1), (1, 1), (0, 0)), mode="reflect")
    halves = np.stack([xp[:, 0:PR], xp[:, ROWS_PER_HALF:ROWS_PER_HALF + PR]],
                      axis=1).reshape(HALVES, PR, PC, C)
    planar = np.ascontiguousarray(
        halves.transpose(0, 3, 1, 2), dtype=np.float16).reshape(
            HALVES, C, PLANE)

    p = np.asarray(params, dtype=np.float32).reshape(NIMG, 7)
    sigma_r = np.clip(p[:, 1], 0.01, 1.0)
    sigma_s = np.clip(p[:, 0], 0.2, 5.0)
    sigma_f = np.clip(p[:, 2], 0.2, 3.0)
    lam = np.clip(p[:, 3], 0.1, 2.0)
    tau = np.clip(p[:, 4], 0.5, 5.0)
    gain = np.clip(p[:, 5], 0.2, 2.0)
    offset = np.clip(p[:, 6], 0.01, 1.0)

    def gauss1d(sig):
        g = np.exp(-0.5 * (np.array([-1.0, 0.0, 1.0], np.float32)[None, :]
                           / sig[:, None]) ** 2)
        return g / g.sum(axis=1, keepdims=True)

    gs = gauss1d(sigma_s)
    gf = gauss1d(sigma_f)
    aE, aC = gs[:, 0], gs[:, 1]
    bE = gf[:, 0]

    kpos = max(abs(float(np.asarray(k))), 1.0)
    gt = gain / tau
    sql2 = np.sqrt(lam * bE)

    pars = np.zeros((NIMG, NPARAM), np.float32)
    pars[:, P_S] = np.sqrt(0.5) / sigma_r
    pars[:, P_LOGE] = np.log(aE * aC)
    pars[:, P_LOGC] = np.log(aE * aE)
    pars[:, P_WSC] = aC * aC
    pars[:, P_BE] = bE
    pars[:, P_1M2BE] = 1.0 - 2.0 * bE
    pars[:, P_IGT] = 1.0 / gt
    pars[:, P_OFFGT] = offset / gt
    pars[:, P_CLIP] = 10.0 / tau
    pars[:, P_KT] = 0.5 * kpos
    pars[:, P_KTB] = -0.5 * kpos * NOISE_THRESH
    pars[:, P_SQL2] = sql2
    pars[:, P_NSQL2] = -sql2
    pars[:, P_NS2] = -0.5 / (sigma_r * sigma_r)
    pars[:, P_HLB] = 0.5 * lam * bE
    pars[:, P_M1] = -1.0
    pars2 = np.repeat(pars, 2, axis=0)  # per half-image

    in_maps = []
    per_core = HALVES // N_CORES
    for c in range(N_CORES):
        sl = slice(c * per_core, (c + 1) * per_core)
        in_maps.append({
            "xpad": np.ascontiguousarray(planar[sl]),
            "pp": np.ascontiguousarray(pars2[sl]),
        })
    return in_maps


def _host_post(results, images, params, chunks):
    cr = ROWS_PER_HALF // chunks
    il = cr * PC  # interior flat length per plane per chunk
    outs = [np.asarray(r["out"], np.float32) for r in results]
    full = np.concatenate(outs, axis=0)  # [1024, chunks, 3, il]
    full = full.reshape(HALVES, chunks, C, cr, PC)[:, :, :, :, 1:65]
    # -> [1024, 32, 64, 3]
    full = full.transpose(0, 1, 3, 4, 2).reshape(NIMG, H, W, C)

    sk = np.concatenate([np.asarray(r["accs"], np.float64)
                         for r in results], axis=0)  # [1024, 2*chunks]
    a_half = sk[:, 0:chunks].sum(axis=1)
    n_half = sk[:, chunks:2 * chunks].sum(axis=1)
    a_img = a_half[0::2] + a_half[1::2]
    n_img = n_half[0::2] + n_half[1::2]
    tau = np.clip(np.asarray(params, np.float32).reshape(NIMG, 7)[:, 4],
                  0.5, 5.0)
    skip = (a_img < MEAN_N * SKIP_THRESH) | (n_img < MEAN_N * SKIP_THRESH / tau)
    if skip.any():
        x = np.asarray(images, np.float32).reshape(NIMG, H, W, C)
        full[skip] = np.clip(x[skip], 1e-5, 1.0)
    return full.reshape(B, PP, H, W, C)


# --------------------------------------------------------------------------
# device program
# --------------------------------------------------------------------------

def build_program(cfg=None):
    import concourse.tile as tile
    from concourse import bacc, mybir
    from contextlib import ExitStack

    cfg = dict(cfg or {})
    F32 = mybir.dt.float32
    F16 = mybir.dt.float16
    ALU = mybir.AluOpType
    AF = mybir.ActivationFunctionType

    repeat = int(cfg.get("repeat", 1))
    CHUNKS = int(cfg.get("chunks", 4))
    CR = ROWS_PER_HALF // CHUNKS
    SR = CR + 2
    FL = SR * PC + BPAD      # field tile length per plane (flat + back pad)
    IL = CR * PC             # interior flat length (rows 1..CR, all 66 cols)
    IOFF = PC                # interior start offset inside a field span

    # engine knobs
    pool_pair = bool(cfg.get("pool_pair", False))
    pool_d2 = int(cfg.get("pool_d2", 0))      # taps whose d2 adds go to Pool
    pool_ptree = bool(cfg.get("pool_ptree", False))
    pool_av = bool(cfg.get("pool_av", False))
    pool_ne0 = bool(cfg.get("pool_ne0", False))
    act_sq = int(cfg.get("act_sq", 4))        # taps whose square runs on ACT
    act_t2 = bool(cfg.get("act_t2", False))
    act_nm = bool(cfg.get("act_nm", True))

    nc = bacc.Bacc("TRN2", target_bir_lowering=False, debug=False)
    xdram = nc.dram_tensor("xpad", [128, C, PLANE], F16,
                           kind="ExternalInput").ap()
    pdram = nc.dram_tensor("pp", [128, NPARAM], F32,
                           kind="ExternalInput").ap()
    odram = nc.dram_tensor("out", [128, CHUNKS, C, IL], F16,
                           kind="ExternalOutput").ap()
    adram = nc.dram_tensor("accs", [128, 2 * CHUNKS], F32,
                           kind="ExternalOutput").ap()

    with tile.TileContext(nc) as tc:
        with ExitStack() as ctx:
            pool = ctx.enter_context(tc.tile_pool(name="main", bufs=1))

            pp = pool.tile([128, NPARAM], F32, tag="pp", bufs=1)
            nc.sync.dma_start(pp[:], pdram[:])

            def par(col):
                return pp[:, col:col + 1]

            for rep in range(repeat):
              xs = pool.tile([128, C, PLANE_T], F16, tag="xs", bufs=2,
                             name=f"xs{rep}")
              nc.gpsimd.memset(xs[:, :, 0:FPAD], 0.25)
              nc.gpsimd.memset(xs[:, :, FPAD + PLANE:PLANE_T], 0.25)
              nc.sync.dma_start(xs[:, :, FPAD:FPAD + PLANE], xdram[:])
              accs = pool.tile([128, 2 * CHUNKS], F32, tag="accs", bufs=2,
                               name=f"accs{rep}")

              for ch in range(CHUNKS):
                base = FPAD + ch * CR * PC   # slab row 0, col 0 flat offset
                sfx = f"{ch}_{rep}"

                def xw(off, ln=FL):
                    # [3, ln] plane-major window of xs at slab offset `off`
                    return xs[:, :, base + off:base + off + ln]

                # ---- tap diff fields, flat [3, FL] ----
                # dW[k]  = x[k-1]  - x[k]   (west neighbor)
                # dN[k]  = x[k-66] - x[k]
                # dNW[k] = x[k-67] - x[k]
                # dNE[k] = x[k-65] - x[k]
                taps = {}
                for nm_, off, tag in [("W", -1, "dWN"), ("N", -PC, "dWN"),
                                      ("NW", -PC - 1, "dC"),
                                      ("NE", -PC + 1, "dC")]:
                    d = pool.tile([128, C, FL], F16, tag=tag,
                                  bufs=4 if tag == "dWN" else 3,
                                  name=f"d{nm_}{sfx}")
                    nc.vector.tensor_tensor(d[:], xw(off), xw(0), ALU.subtract)
                    taps[nm_] = d

                # ---- squares, channel sums -> d2, exp -> ck ----
                d2EN = pool.tile([128, 2, FL], F16, tag="d2", bufs=4,
                                 name=f"d2EN{sfx}")
                d2C = pool.tile([128, 2, FL], F16, tag="d2", bufs=4,
                                name=f"d2C{sfx}")
                for ti, (nm_, d2t, g) in enumerate(
                        [("W", d2EN, 0), ("N", d2EN, 1),
                         ("NW", d2C, 0), ("NE", d2C, 1)]):
                    dt_ = taps[nm_]
                    sq = pool.tile([128, C, FL], F16, tag="sq", bufs=3,
                                   name=f"sq{ti}_{sfx}")
                    if ti < act_sq:
                        nc.scalar.activation(sq[:], dt_[:], AF.Square,
                                             scale=par(P_S))
                    else:
                        nc.vector.tensor_tensor(sq[:], dt_[:], dt_[:],
                                                ALU.mult)
                    eng = nc.gpsimd if ti < pool_d2 else nc.vector
                    d2a = pool.tile([128, FL], F16, tag="d2a", bufs=2,
                                    name=f"d2a{ti}_{sfx}")
                    eng.tensor_tensor(d2a[:], sq[:, 0], sq[:, 1], ALU.add)
                    eng.tensor_tensor(d2t[:, g], d2a[:], sq[:, 2], ALU.add)

                esc = -1.0 if act_sq else par(P_NS2)
                ckEN = pool.tile([128, 2, FL], F16, tag="ck", bufs=4,
                                 name=f"ckEN{sfx}")
                nc.scalar.activation(ckEN[:], d2EN[:], AF.Exp,
                                     bias=par(P_LOGE), scale=esc)
                ckC = pool.tile([128, 2, FL], F16, tag="ck", bufs=4,
                                name=f"ckC{sfx}")
                nc.scalar.activation(ckC[:], d2C[:], AF.Exp,
                                     bias=par(P_LOGC), scale=esc)

                # ---- prod, dif = prod_I - prod_M, pair ----
                # M offset in flat coords = -(tap offset)
                TAPW = [("W", ckEN, 0, 1), ("N", ckEN, 1, PC),
                        ("NW", ckC, 0, PC + 1), ("NE", ckC, 1, PC - 1)]
                difs = []
                pairs = []
                for ti, (nm_, ckt, g, mo) in enumerate(TAPW):
                    dt_ = taps[nm_]
                    prod = pool.tile([128, C, FL], F16, tag="prod", bufs=4,
                                     name=f"pr{ti}_{sfx}")
                    nc.vector.tensor_tensor(
                        prod[:], dt_[:],
                        ckt[:, g].unsqueeze(1).broadcast_to([128, C, FL]),
                        ALU.mult)
                    dif = pool.tile([128, C, IL], F16, tag="dif", bufs=4,
                                    name=f"dif{ti}_{sfx}")
                    nc.vector.tensor_tensor(
                        dif[:], prod[:, :, IOFF:IOFF + IL],
                        prod[:, :, IOFF + mo:IOFF + mo + IL], ALU.subtract)
                    difs.append(dif)
                    pair = pool.tile([128, IL], F16, tag="pair", bufs=6,
                                     name=f"pai{ti}_{sfx}")
                    peng = nc.gpsimd if pool_pair else nc.vector
                    peng.tensor_tensor(
                        pair[:], ckt[:, g, IOFF:IOFF + IL],
                        ckt[:, g, IOFF + mo:IOFF + mo + IL], ALU.add)
                    pairs.append(pair)

                n01 = pool.tile([128, C, IL], F16, tag="nacc", bufs=3,
                                name=f"n01{sfx}")
                nc.vector.tensor_tensor(n01[:], difs[0][:], difs[1][:],
                                        ALU.add)
                n23 = pool.tile([128, C, IL], F16, tag="nacc", bufs=3,
                                name=f"n23{sfx}")
                nc.vector.tensor_tensor(n23[:], difs[2][:], difs[3][:],
                                        ALU.add)
                nacc = pool.tile([128, C, IL], F16, tag="nacc", bufs=3,
                                 name=f"nacc{sfx}")
                nc.vector.tensor_tensor(nacc[:], n01[:], n23[:], ALU.add)

                teng = nc.gpsimd if pool_ptree else nc.vector
                p01 = pool.tile([128, IL], F16, tag="ptree", bufs=3,
                                name=f"p01{sfx}")
                teng.tensor_tensor(p01[:], pairs[0][:], pairs[1][:], ALU.add)
                p23 = pool.tile([128, IL], F16, tag="ptree", bufs=3,
                                name=f"p23{sfx}")
                teng.tensor_tensor(p23[:], pairs[2][:], pairs[3][:], ALU.add)
                q = pool.tile([128, IL], F16, tag="ptree", bufs=3,
                              name=f"q{sfx}")
                teng.tensor_tensor(q[:], p01[:], p23[:], ALU.add)
                sacc = pool.tile([128, IL], F32, tag="sacc", bufs=2,
                                 name=f"sacc{sfx}")
                nc.vector.tensor_scalar(sacc[:], q[:], par(P_WSC), None,
                                        ALU.add)
                rS = pool.tile([128, IL], F32, tag="rS", bufs=2,
                               name=f"rS{sfx}")
                nc.vector.reciprocal_approx_fast(rS[:], sacc[:])
                tT = pool.tile([128, C, IL], F16, tag="tT", bufs=2,
                               name=f"tT{sfx}")
                nc.vector.tensor_tensor(
                    tT[:], nacc[:],
                    rS[:].unsqueeze(1).broadcast_to([128, C, IL]), ALU.mult)

                # ---- gaussian detail from dW/dN ----
                # H[k] = dW[k+2] - dW[k+1]  (= -Lx at flat pos k+1)
                dW, dN = taps["W"], taps["N"]
                Hf = pool.tile([128, C, FL], F16, tag="H", bufs=2,
                               name=f"H{sfx}")
                nc.vector.tensor_tensor(
                    Hf[:, :, 0:FL - 1], dW[:, :, 1:FL], dW[:, :, 0:FL - 1],
                    ALU.subtract)
                V = pool.tile([128, C, IL], F16, tag="s1", bufs=5,
                              name=f"V{sfx}")
                nc.vector.tensor_tensor(
                    V[:], dN[:, :, IOFF + PC:IOFF + PC + IL],
                    dN[:, :, IOFF:IOFF + IL], ALU.subtract)
                av = pool.tile([128, C, IL], F16, tag="s1", bufs=5,
                               name=f"av{sfx}")
                (nc.gpsimd if pool_av else nc.vector).tensor_tensor(
                    av[:], Hf[:, :, IOFF - PC:IOFF - PC + IL],
                    Hf[:, :, IOFF + PC:IOFF + PC + IL], ALU.add)
                z1 = pool.tile([128, C, IL], F16, tag="s1", bufs=5,
                               name=f"z1{sfx}")
                nc.vector.affine_then_add(z1[:], Hf[:, :, IOFF:IOFF + IL],
                                          V[:], par(P_1M2BE), 0.0)
                inner = pool.tile([128, C, IL], F16, tag="inner", bufs=2,
                                  name=f"inner{sfx}")
                nc.vector.affine_then_add(inner[:], av[:], z1[:],
                                          par(P_BE), 0.0)
                # zero the two pad columns so accumulators stay exact
                iv = inner[:].rearrange("p c (r s) -> p c r s", s=PC)
                nc.gpsimd.memset(iv[:, :, :, 0:1], 0.0)
                nc.gpsimd.memset(iv[:, :, :, 65:66], 0.0)

                # ---- noise / masks ----
                adet = pool.tile([128, C, IL], F16, tag="s1", bufs=5,
                                 name=f"adet{sfx}")
                nc.scalar.activation(adet[:], inner[:], AF.Abs,
                                     scale=par(P_BE),
                                     accum_out=accs[:, ch:ch + 1])
                th = pool.tile([128, C, IL], F16, tag="s1", bufs=5,
                               name=f"th{sfx}")
                nc.scalar.activation(th[:], adet[:], AF.Tanh,
                                     bias=par(P_KTB), scale=par(P_KT))
                th1 = pool.tile([128, C, IL], F16, tag="th1", bufs=2,
                                name=f"th1{sfx}")
                nc.vector.tensor_scalar(th1[:], th[:], par(P_HLB),
                                        par(P_HLB), ALU.mult, ALU.add)
                d1 = pool.tile([128, C, IL], F32, tag="d32", bufs=2,
                               name=f"d1{sfx}")
                nc.scalar.activation(d1[:], xw(IOFF, IL), AF.Identity,
                                     scale=par(P_IGT), bias=par(P_OFFGT))
                r1 = pool.tile([128, C, IL], F32, tag="d32", bufs=2,
                               name=f"r1{sfx}")
                nc.vector.reciprocal_approx_fast(
                    r1[:].rearrange("p a b -> p (a b)"),
                    d1[:].rearrange("p a b -> p (a b)"))
                ne0 = pool.tile([128, C, IL], F16, tag="s1", bufs=5,
                                name=f"ne0{sfx}")
                (nc.gpsimd if pool_ne0 else nc.vector).tensor_tensor(
                    ne0[:], adet[:], r1[:], ALU.mult)
                neq = pool.tile([128, C, IL], F16, tag="s1", bufs=5,
                                name=f"neq{sfx}")
                nc.vector.tensor_scalar(
                    neq[:], ne0[:], par(P_CLIP), None, ALU.min, ALU.add,
                    accum_out=accs[:, CHUNKS + ch:CHUNKS + ch + 1])
                sqn = pool.tile([128, C, IL], F16, tag="s1", bufs=5,
                                name=f"sqn{sfx}")
                nc.scalar.activation(sqn[:], neq[:], AF.Square)
                ee = pool.tile([128, C, IL], F16, tag="s1", bufs=5,
                               name=f"ee{sfx}")
                nc.scalar.activation(ee[:], sqn[:], AF.Exp, scale=-1.0)
                nm = pool.tile([128, C, IL], F16, tag="s1", bufs=5,
                               name=f"nm{sfx}")
                nc.scalar.activation(nm[:], ee[:], AF.Square, scale=1.0,
                                     bias=par(P_M1))
                s3 = pool.tile([128, C, IL], F16, tag="s1", bufs=5,
                               name=f"s3{sfx}")
                nc.vector.tensor_tensor(s3[:], th1[:], nm[:], ALU.mult)
                sharp = pool.tile([128, C, IL], F16, tag="s1", bufs=5,
                                  name=f"sh{sfx}")
                nc.vector.tensor_tensor(sharp[:], s3[:], inner[:], ALU.mult)

                # ---- combine + clip + store ----
                t3 = pool.tile([128, C, IL], F16, tag="s1", bufs=5,
                               name=f"t3{sfx}")
                nc.vector.tensor_tensor(t3[:], tT[:], sharp[:], ALU.add)
                o3a = pool.tile([128, C, IL], F16, tag="s1", bufs=5,
                                name=f"o3a{sfx}")
                nc.vector.tensor_tensor(o3a[:], xw(IOFF, IL), t3[:], ALU.add)
                o3 = pool.tile([128, C, IL], F16, tag="o3", bufs=2,
                               name=f"o3{sfx}")
                nc.vector.tensor_scalar(o3[:], o3a[:], 1e-5, 1.0,
                                        ALU.max, ALU.min)
                nc.sync.dma_start(odram[:, ch], o3[:])

              nc.sync.dma_start(adram[:], accs[:])

    nc.compile()
    return nc


def _get_program(cfg=None):
    key = tuple(sorted((cfg or {}).items()))
    if key not in _CACHE:
        _CACHE[key] = build_program(cfg)
    return _CACHE[key]


# --------------------------------------------------------------------------
# entry point
# --------------------------------------------------------------------------

DEFAULT_CFG = {}


def kernel(images, params, k):
    from concourse.bass_utils import run_bass_kernel_spmd

    cfg = dict(DEFAULT_CFG)
    nc = _get_program(cfg)
    in_maps = _host_prep(np.asarray(images), np.asarray(params), np.asarray(k))
    res = run_bass_kernel_spmd(nc, in_maps, list(range(N_CORES)))
    chunks = int(cfg.get("chunks", 4))
    return _host_post(res.results, images, params, chunks).astype(np.float32)
